# revision 21
# baseline (speedup 1.0000x reference)
"""Trainium2 Bass kernel for BilinearCategoricalNet.

  h1 = relu(relu(x1 @ m1_w1.T + m1_b1) @ m1_w2.T + m1_b2)      [B, H]
  h2 = same for x2 / m2
  o1 = einsum('bh,oph->bpo', h1, mll1_w) + mll1_b.T            [B, P, O]
  o2 = same for h2 / mll2
  logits = sum_p o1 * o2                                       [B, O]

Strategy: data-parallel over 8 cores (1024 rows each), weights replicated.
All matmul operands in float16 (same 1 row/cycle PE rate as float32r and
the same ~11-bit mantissa, but half the DMA/SBUF traffic). Activations
kept feature-major [h, b] so every contraction has its contraction dim on
SBUF partitions. The MLL stage emits batch-major [b, (o,p)] PSUM tiles so
sum_p becomes a DVE free-axis segmented reduce. MLL biases are folded
into precomputed corrections:
  logits = sum_p a*c + h1 @ v1.T + h2 @ v2.T + c0
where a/c are the bias-free MLL outputs, v1[o] = sum_p mll2_b[o,p]*mll1_w[o,p],
v2[o] = sum_p mll1_b[o,p]*mll2_w[o,p], c0[o] = mll1_b[o] . mll2_b[o].

Scheduling notes (vs the fp32r baseline):
- DMA issues cost ~0.6-0.9us of engine-queue time each and each queue's
  ring serves transfers FIFO, so MLP loads round-robin across the three
  DMA-capable queues (sync/gpsimd/scalar), gating loads (x+w1) first.
  The scalar queue never gets a DMA before the relus that recycle PSUM.
- A PE warm-up burst runs during the DMA lead-in so the HAM clock gate
  (1.2GHz cold -> 2.4GHz after ~3.4us of activity) opens before real work.
- MLL weight chunks triple-buffer in the persistent pool (a post-MLP
  pool would WAR-block the first loads until the MLP finishes) and
  stream as per-hc DMAs over all three queues; chunks 0/1 prefetch
  during the MLP entirely via sync (scalar is still draining relus).
- One shared 8-buf PSUM tag lets the PE run up to 8 groups ahead of the
  ACT/DVE consumers.
- Per (chunk, bt): net-2 matmuls run first so the PSUM->SBUF bounce copy
  (DVE reads only one PSUM operand) overlaps net-1's matmuls.
- The final corr-add + store is folded into the last chunk's bt loop;
  the last store is split across two queues (a 64KB store rides one
  ~22GB/s ring for ~2.9us and would gate the shutdown drain).
"""
import sys

sys.path.insert(0, "/opt/trn_rl_repo")

import numpy as np

B = 8192
NCORES = 8
BL = B // NCORES          # 1024 rows per core
NI = 512                  # input features
H = 1024                  # hidden
O = 128                   # num outputs
P = 64                    # pre-bilinear
OP = O * P                # 8192 flattened (o, p), o-major
KC1 = NI // 128           # 4 k-chunks, layer 1
HC = H // 128             # 8 h-chunks
BCH = BL // 512           # 2 batch chunks of 512 (MLP free dim)
BT = BL // 128            # 8 batch tiles of 128 (MLL stationary dim)
CH = OP // 512            # 16 (o,p)-chunks of 512 (= 8 o's each)

_CACHED = None


def _build(phase=3, n_chunks=CH):
    """phase: 1=MLPs only, 2=+corrections, 3=full (+MLL)."""
    import concourse.bacc as bacc
    import concourse.mybir as mybir
    from concourse.tile import TileContext

    f32 = mybir.dt.float32
    f16 = mybir.dt.float16
    Relu = mybir.ActivationFunctionType.Relu
    Add = mybir.AluOpType.add
    AX = mybir.AxisListType.X

    nc = bacc.Bacc("TRN2", target_bir_lowering=False, debug=False,
                   num_devices=NCORES)

    def din(name, shape, dt=f16):
        return nc.dram_tensor(name, shape, dt, kind="ExternalInput").ap()

    xT = [din("xT1", [NI, BL]), din("xT2", [NI, BL])]
    w1T = [din("w1T_1", [NI, H]), din("w1T_2", [NI, H])]
    w2T = [din("w2T_1", [H, H]), din("w2T_2", [H, H])]
    b1 = [din("b1_1", [128, HC], f32), din("b1_2", [128, HC], f32)]
    b2 = [din("b2_1", [128, HC], f32), din("b2_2", [128, HC], f32)]
    mllT = [din("mllT1", [H, OP]), din("mllT2", [H, OP])]
    vT = [din("v1T", [H, O]), din("v2T", [H, O])]
    c0 = din("c0", [1, O])
    ones = din("ones", [1, 512])
    ident = din("ident", [128, 128], f32)
    out = nc.dram_tensor("out", [BL, O], f32, kind="ExternalOutput").ap()

    with TileContext(nc) as tc:
        with tc.tile_pool(name="persist", bufs=1) as pp, \
             tc.tile_pool(name="ps", bufs=1, space="PSUM") as ps:
            # long-lived small tensors; biases load off the sync queue so
            # the x/w gating DMAs lead
            b1_sb = [pp.tile([128, HC], f32, name=f"b1sb{n}") for n in range(2)]
            b2_sb = [pp.tile([128, HC], f32, name=f"b2sb{n}") for n in range(2)]
            # final MLP outputs, feature-major [h, b] — live through MLL phase
            f_sb = [[pp.tile([128, BL], f16, name=f"f{n}_{m}") for m in range(HC)]
                    for n in range(2)]
            logits_sb = pp.tile([128, BT, O], f32, name="logits_sb")
            corr_sb = pp.tile([128, BT, O], f32, name="corr_sb")
            v_sb = [pp.tile([128, HC, O], f16, name=f"vsb{n}") for n in range(2)]
            c0_sb = pp.tile([1, O], f16, name="c0sb")
            ones_sb = pp.tile([1, 512], f16, name="onessb")
            ident_sb = pp.tile([128, 128], f32, name="identsb")
            corr_om = pp.tile([128, BL], f32, name="corr_om")
            # MLL weight chunks triple-buffer in the persistent pool: a
            # post-MLP pool would overlap the MLP tiles' SBUF space and the
            # WAR dependency would block the first chunk loads until the
            # MLP finishes
            mt_slots = [[pp.tile([128, HC, 512], f16, name=f"mt{n}_{s}")
                         for s in range(3)] for n in range(2)]

            # ---------------- MLP phase (per net, shared slots) -------------
            with tc.tile_pool(name="mlp", bufs=1) as mp:
                # PE warm-up: the HAM clock gate starts at 1.2GHz and only
                # opens to 2.4GHz after ~3.4us of sustained PE activity.
                # Burn that window on dummy matmuls while the first DMAs are
                # still in flight.
                warm = mp.tile([128, 128], f16, name="warm", tag="warm")
                nc.vector.memset(warm, 0.0)
                for _ in range(44):
                    pw = ps.tile([128, 128], f32, name="pw", tag="acc", bufs=8)
                    nc.tensor.matmul(pw, warm, warm, start=True, stop=True)
                for n in range(2):
                    # 2*KC1 bufs: net 2's x/w1 prefetch fully while net 1
                    # still reads the old slots
                    x_t = [mp.tile([128, BL], f16, name=f"x_{kc}", tag="x_t",
                                   bufs=2 * KC1) for kc in range(KC1)]
                    w1_t = [mp.tile([128, H], f16, name=f"w1_{kc}", tag="w1_t",
                                    bufs=2 * KC1) for kc in range(KC1)]
                    w2s = mp.tile([128, HC, H], f16, name="w2s", tag="w2s",
                                  bufs=2)
                    # Each DMA queue's ring moves ~90GB/s and serves its
                    # transfers FIFO, so balance bytes evenly across all
                    # three DMA-capable queues, gating loads (x+w1) first.
                    rr = [nc.sync, nc.gpsimd, nc.scalar]
                    ri = 0

                    def issue(out_ap, in_ap):
                        nonlocal ri
                        rr[ri % 3].dma_start(out=out_ap, in_=in_ap)
                        ri += 1

                    for kc in range(KC1):
                        issue(x_t[kc], xT[n][kc * 128:(kc + 1) * 128, :])
                        issue(w1_t[kc], w1T[n][kc * 128:(kc + 1) * 128, :])
                    nc.gpsimd.dma_start(out=b1_sb[n], in_=b1[n])
                    nc.gpsimd.dma_start(out=b2_sb[n], in_=b2[n])
                    # column-halves, low half first: L2's first 4
                    # m-groups gate on 1MB instead of 2MB
                    for mh in range(2):
                        for kc in range(HC):
                            issue(w2s[:, kc, mh * 512:(mh + 1) * 512],
                                  w2T[n][kc * 128:(kc + 1) * 128,
                                         mh * 512:(mh + 1) * 512])
                    h_t = [mp.tile([128, BL], f16, name=f"h_{m}", tag=f"h_{m}")
                           for m in range(HC)]
                    # layer 1: h[m] = relu(w1.T @ x + b1)
                    for m in range(HC):
                        for bc in range(BCH):
                            pm = ps.tile([128, 512], f32, name="pm", tag="acc",
                                         bufs=8)
                            for kc in range(KC1):
                                nc.tensor.matmul(
                                    pm,
                                    w1_t[kc][:, m * 128:(m + 1) * 128],
                                    x_t[kc][:, bc * 512:(bc + 1) * 512],
                                    start=(kc == 0), stop=(kc == KC1 - 1))
                            nc.scalar.activation(
                                h_t[m][:, bc * 512:(bc + 1) * 512], pm, Relu,
                                bias=b1_sb[n][:, m:m + 1])
                    # layer 2: f[m] = relu(w2.T @ h + b2)
                    for m in range(HC):
                        for bc in range(BCH):
                            pm = ps.tile([128, 512], f32, name="pm", tag="acc",
                                         bufs=8)
                            for kc in range(HC):
                                nc.tensor.matmul(
                                    pm,
                                    w2s[:, kc, m * 128:(m + 1) * 128],
                                    h_t[kc][:, bc * 512:(bc + 1) * 512],
                                    start=(kc == 0), stop=(kc == HC - 1))
                            nc.scalar.activation(
                                f_sb[n][m][:, bc * 512:(bc + 1) * 512], pm, Relu,
                                bias=b2_sb[n][:, m:m + 1])

            if phase == 1:
                for m in range(HC):
                    nc.sync.dma_start(
                        out=out[m * 128:(m + 1) * 128, :],
                        in_=f_sb[0][m].bitcast(f32)[:, 0:O])

            # correction inputs (small) + first MLL weight chunk prefetch;
            # all issued while the MLP phase still computes
            for n in range(2):
                nc.gpsimd.dma_start(
                    out=v_sb[n], in_=vT[n].rearrange("(hc p) o -> p hc o", p=128))
            nc.gpsimd.dma_start(out=c0_sb, in_=c0)
            nc.gpsimd.dma_start(out=ones_sb, in_=ones)
            nc.gpsimd.dma_start(out=ident_sb, in_=ident)

            with tc.tile_pool(name="mll", bufs=1) as lp:
                def load_chunk(c):
                    # one DMA per h-chunk, alternating queues: cheap issues
                    # (a whole-tile 3D DMA costs a ~4us issue slice) and
                    # parallel transfer across DMA engines
                    m_t = [mt_slots[n][c % 3] for n in range(2)]
                    for n in range(2):
                        src_ap = mllT[n].rearrange(
                            "(hc p) f -> p hc f",
                            p=128)[:, :, c * 512:(c + 1) * 512]
                        for hc in range(HC):
                            # chunks 0/1 prefetch entirely via sync: the
                            # scalar queue is still draining MLP relus and
                            # would issue too late. Steady-state chunks
                            # spread over all three queues — two rings sit
                            # right at the 2MB/27.6us chunk cadence edge.
                            eng = (nc.sync if c < 2 else
                                   [nc.sync, nc.gpsimd,
                                    nc.scalar][(n * HC + hc) % 3])
                            eng.dma_start(out=m_t[n][:, hc, :],
                                          in_=src_ap[:, hc, :])
                    return m_t

                nxt = None
                nxt2 = None
                if phase >= 3 and n_chunks > 0:
                    nxt = load_chunk(0)
                    if n_chunks > 1:
                        nxt2 = load_chunk(1)

                # ---------- corrections: h1@v1.T + h2@v2.T + c0 -------------
                # computed o-major (v stationary, N=512) then PE-transposed to
                # batch-major — N=128 f-stationary matmuls are LDWEIGHTS-bound.
                for bc in range(BCH if phase >= 2 else 0):
                    pc = ps.tile([128, 512], f32, name="pc", tag="acc", bufs=8)
                    first = True
                    for n in range(2):
                        for hc in range(HC):
                            nc.tensor.matmul(
                                pc, v_sb[n][:, hc, :],
                                f_sb[n][hc][:, bc * 512:(bc + 1) * 512],
                                start=first, stop=False)
                            first = False
                    nc.tensor.matmul(pc, c0_sb, ones_sb, start=False, stop=True)
                    nc.vector.tensor_copy(corr_om[:, bc * 512:(bc + 1) * 512], pc)
                def emit_pt():
                    # deferred so the corr_om DVE copy hides behind the
                    # first MLL matmul group instead of stalling the PE
                    for bt2 in range(BT):
                        pt = ps.tile([128, O], f32, name="pt", tag="acc",
                                     bufs=8)
                        nc.tensor.transpose(
                            pt, corr_om[:, bt2 * 128:(bt2 + 1) * 128],
                            ident_sb)
                        nc.vector.tensor_copy(corr_sb[:, bt2, :], pt)

                if phase == 2 or (phase >= 2 and n_chunks == 0):
                    emit_pt()
                if phase == 2:
                    for bt in range(BT):
                        nc.sync.dma_start(out=out[bt * 128:(bt + 1) * 128, :],
                                          in_=corr_sb[:, bt, :])

                # ---------- MLL phase: chunk-outer, btile-inner -------------
                store_eng = None
                for c in range(n_chunks if phase >= 3 else 0):
                    m_t = nxt
                    nxt = nxt2
                    nxt2 = load_chunk(c + 2) if c + 2 < n_chunks else None
                    last = (c == n_chunks - 1)
                    for bt in range(BT):
                        # the very last bt runs in 256-col halves so the
                        # first half's DVE mul/reduce hides behind the
                        # second half's matmuls, shortening the tail
                        parts = ([(0, 256), (256, 512)]
                                 if last and bt == BT - 1 else [(0, 512)])
                        # net 2 first: its PSUM->SBUF bounce copy (DVE can
                        # read only one PSUM operand) overlaps net 1's group
                        o2_parts = []
                        for lo, hi in parts:
                            pr1 = ps.tile([128, hi - lo], f32, name="pr1",
                                          tag="acc", bufs=8)
                            for hc in range(HC):
                                nc.tensor.matmul(
                                    pr1,
                                    f_sb[1][hc][:, bt * 128:(bt + 1) * 128],
                                    m_t[1][:, hc, lo:hi],
                                    start=(hc == 0), stop=(hc == HC - 1))
                            o2_sb = lp.tile([128, hi - lo], f16, name="o2_sb",
                                            tag="o2_sb", bufs=3)
                            nc.vector.tensor_copy(o2_sb, pr1)
                            o2_parts.append(o2_sb)
                        if c == 0 and bt == 0 and phase >= 2:
                            emit_pt()
                        for (lo, hi), o2_sb in zip(parts, o2_parts):
                            pr0 = ps.tile([128, hi - lo], f32, name="pr0",
                                          tag="acc", bufs=8)
                            for hc in range(HC):
                                nc.tensor.matmul(
                                    pr0,
                                    f_sb[0][hc][:, bt * 128:(bt + 1) * 128],
                                    m_t[0][:, hc, lo:hi],
                                    start=(hc == 0), stop=(hc == HC - 1))
                            prod = lp.tile([128, hi - lo], f16, name="prod",
                                           tag="prod", bufs=4)
                            nc.vector.tensor_mul(prod, pr0, o2_sb)
                            nc.vector.tensor_reduce(
                                logits_sb[:, bt,
                                          c * 8 + lo // P:c * 8 + hi // P],
                                prod.rearrange("p (o q) -> p o q", q=P),
                                axis=AX, op=Add)
                        if last:
                            o_sb = lp.tile([128, O], f32, name="o_sb",
                                           tag="o_sb", bufs=2)
                            nc.vector.tensor_add(o_sb, logits_sb[:, bt, :],
                                                 corr_sb[:, bt, :])
                            if bt < BT - 1:
                                eng = nc.sync if bt % 2 == 0 else nc.scalar
                                eng.dma_start(
                                    out=out[bt * 128:(bt + 1) * 128, :],
                                    in_=o_sb)
                            else:
                                # the last store gates the shutdown drain:
                                # split it across two queues to halve the
                                # ~2.9us single-ring transfer
                                nc.sync.dma_start(
                                    out=out[bt * 128:bt * 128 + 64, :],
                                    in_=o_sb[0:64, :])
                                nc.scalar.dma_start(
                                    out=out[bt * 128 + 64:(bt + 1) * 128, :],
                                    in_=o_sb[64:128, :])

    nc.compile()
    return nc


def _get_nc():
    global _CACHED
    if _CACHED is None:
        _CACHED = _build()
    return _CACHED


def _prep_shared(m1_w1, m1_b1, m1_w2, m1_b2, m2_w1, m2_b1, m2_w2, m2_b2,
                 mll1_w, mll1_b, mll2_w, mll2_b):
    """Host-side weight layouts, shared by all cores."""
    f = np.float32
    h = np.float16
    d = {}
    d["w1T_1"] = np.ascontiguousarray(m1_w1.T).astype(h)
    d["w1T_2"] = np.ascontiguousarray(m2_w1.T).astype(h)
    d["w2T_1"] = np.ascontiguousarray(m1_w2.T).astype(h)
    d["w2T_2"] = np.ascontiguousarray(m2_w2.T).astype(h)
    d["b1_1"] = np.ascontiguousarray(m1_b1.reshape(HC, 128).T).astype(f)
    d["b1_2"] = np.ascontiguousarray(m2_b1.reshape(HC, 128).T).astype(f)
    d["b2_1"] = np.ascontiguousarray(m1_b2.reshape(HC, 128).T).astype(f)
    d["b2_2"] = np.ascontiguousarray(m2_b2.reshape(HC, 128).T).astype(f)
    # [O, P, H] -> [H, O*P] with o-major flattened columns
    d["mllT1"] = np.ascontiguousarray(
        mll1_w.transpose(2, 0, 1).reshape(H, OP)).astype(h)
    d["mllT2"] = np.ascontiguousarray(
        mll2_w.transpose(2, 0, 1).reshape(H, OP)).astype(h)
    v1 = np.einsum("op,oph->oh", mll2_b.astype(np.float64),
                   mll1_w.astype(np.float64))
    v2 = np.einsum("op,oph->oh", mll1_b.astype(np.float64),
                   mll2_w.astype(np.float64))
    d["v1T"] = np.ascontiguousarray(v1.T).astype(h)
    d["v2T"] = np.ascontiguousarray(v2.T).astype(h)
    d["c0"] = (mll1_b.astype(np.float64) *
               mll2_b.astype(np.float64)).sum(axis=1)[None, :].astype(h)
    d["ones"] = np.ones((1, 512), dtype=h)
    d["ident"] = np.eye(128, dtype=f)
    return d


def kernel(x_1, x_2, m1_w1, m1_b1, m1_w2, m1_b2, m2_w1, m2_b1, m2_w2, m2_b2,
           mll1_w, mll1_b, mll2_w, mll2_b):
    from concourse.bass_utils import run_bass_kernel_spmd

    nc = _get_nc()
    shared = _prep_shared(np.asarray(m1_w1), np.asarray(m1_b1),
                          np.asarray(m1_w2), np.asarray(m1_b2),
                          np.asarray(m2_w1), np.asarray(m2_b1),
                          np.asarray(m2_w2), np.asarray(m2_b2),
                          np.asarray(mll1_w), np.asarray(mll1_b),
                          np.asarray(mll2_w), np.asarray(mll2_b))
    x_1 = np.asarray(x_1, dtype=np.float32)
    x_2 = np.asarray(x_2, dtype=np.float32)
    in_maps = []
    for c in range(NCORES):
        sl = slice(c * BL, (c + 1) * BL)
        m = dict(shared)
        m["xT1"] = np.ascontiguousarray(x_1[sl].T).astype(np.float16)
        m["xT2"] = np.ascontiguousarray(x_2[sl].T).astype(np.float16)
        in_maps.append(m)
    res = run_bass_kernel_spmd(nc, in_maps, list(range(NCORES)))
    return np.concatenate([res.results[c]["out"] for c in range(NCORES)],
                          axis=0)
